# revision 29
# baseline (speedup 1.0000x reference)
"""Two-layer GCN (PyG GCNConv x2, eval mode) on 8 Trainium2 NeuronCores.

out = S @ (relu(S @ (x@W1) + b1) @ W2) + b2,  S = D^-1/2 (A+I) D^-1/2

v3 design (destination-sharded, direct shared-table writes):
  - 50000 nodes sharded 6250/core (padded to 6272 = 49 blocks of 128);
    per core, destinations are permuted into blocks balanced by in-degree
    (host permutation, undone on the way out).
  - Host supplies x.T pre-scaled by dinv ([Fin, 6272] bf16), so phase A
    (h' = dinv*(x@W1)) is one matmul per block, written STRAIGHT into the
    shared-DRAM gather table at a per-core DynSlice row offset (core id
    loaded into a register at runtime). No AllGather collective copies:
    a tiny flag AllGather per table half is the cross-core barrier, with
    a 2-byte read-back per store batch forcing the RAW on the store DMAs.
  - Each core's own table shard stays resident in SBUF (hall/hall2), so
    the self-loop "own rows" matmul needs no DMA load.
  - Table rows are ROWW*256B (128 bf16 cols + optional pad). int16 gather
    indices force a lo/hi table-half split at <=32768 rows per half.
  - Aggregation per 128-dest block: dma_gather source rows (<=1024 idx
    per call, round-robined over 4 SWDGE queues), one-hot selectors built
    1 op/block via broadcast-AP is_equal, matmul(lhsT=onehot, rhs=msgs)
    accumulating in PSUM; an identity-selector matmul adds the block's
    own rows (the reference's self-loops never touch the gather path).
  - Gather emission is software-pipelined: lo(g+1) issues before hi(g) so
    a hi call waiting on the second table half never starves the queue.
  - Layer 1 tail: h2pre = dinv*relu(dinv*G) = relu(dinv^2*G) in one
    ScalarE op (b1==0) into hall2, batch-stored to the layer-2 table;
    layer 2 applies W2 after aggregation: out = dinv*(G2)@W2 + b2,
    emitted feature-major and transposed on host.
"""

import math
from contextlib import ExitStack

import numpy as np

NC = 8
P = 128
GROUP = 4  # dest blocks per gather buffer group
MAXCALL = 8  # tiles per dma_gather call (1024 idx ucode limit)
NQUEUES = 4
PAD_DEST = 200  # destid for padding edges; never matches iota 0..127
ROWW = 1  # table row width multiplier: 1 => 256B rows, 2 => 512B rows
STORE_BATCH = 7  # dest blocks per table-store DMA (49 = 7*7)


def _pack_idx(v: np.ndarray) -> np.ndarray:
    """[T*128] int -> [128, 8T] int16 in dma_gather's wrap-16 layout,
    replicated over the 8 gpsimd cores (element i lives at [i%16, i//16])."""
    assert v.size % P == 0
    a = v.reshape(-1, 16).T.astype(np.int16)  # [16, 8T]
    return np.tile(a, (8, 1))  # [128, 8T]


def _balance_blocks(weights: np.ndarray, nblk: int) -> np.ndarray:
    """Assign len(weights) items into nblk blocks of <=128, balancing block
    weight sums. Returns pos[i] = block*128 + slot."""
    import heapq

    n = weights.size
    order = np.argsort(-weights, kind="stable")
    loads = np.zeros(nblk, dtype=np.int64)
    fill = np.zeros(nblk, dtype=np.int64)
    cap = np.full(nblk, P, dtype=np.int64)
    cap[nblk - 1] = n - (nblk - 1) * P  # last block holds the remainder
    pos = np.empty(n, dtype=np.int64)
    heap = [(0, b) for b in range(nblk)]
    heapq.heapify(heap)
    for i in order:
        while True:
            load, b = heapq.heappop(heap)
            if fill[b] < cap[b]:
                break
        pos[i] = b * P + fill[b]
        fill[b] += 1
        loads[b] = load + weights[i]
        if fill[b] < cap[b]:
            heapq.heappush(heap, (int(loads[b]), b))
    return pos


def _preprocess(x, edge_index, W1, b1, W2, b2):
    import ml_dtypes

    N, Fin = x.shape
    Fh = W1.shape[1]
    Fout = W2.shape[1]
    assert N % NC == 0
    NPC = N // NC
    NBLK = math.ceil(NPC / P)
    NBP = NBLK * P  # padded rows per core in the gather tables

    row = np.asarray(edge_index[0], dtype=np.int64)
    col = np.asarray(edge_index[1], dtype=np.int64)

    # degrees include the self-loop the reference appends to every node
    deg = (np.bincount(col, minlength=N) + 1).astype(np.float64)
    dinv = (1.0 / np.sqrt(deg)).astype(np.float32)

    # per-core balanced permutation of destination slots by in-edge count
    cnt_in = np.bincount(col, minlength=N)
    pos_in_core = np.empty(N, dtype=np.int64)
    node_at = np.empty((NC, NPC), dtype=np.int64)
    for c in range(NC):
        w = cnt_in[c * NPC : (c + 1) * NPC]
        p = _balance_blocks(w, NBLK)
        pos_in_core[c * NPC : (c + 1) * NPC] = p
        node_at[c][p] = np.arange(NPC) + c * NPC

    # two table halves (int16 gather index limit: NC*ln <= 32768 rows);
    # half k covers core-local padded rows [r0,r1) (block-aligned); the
    # table holds the 8 cores' slices consecutively: node (c, p, half k)
    # lives at table-k row c*len_k + (p - r0_k).
    groups0 = [list(range(g, min(g + GROUP, NBLK))) for g in range(0, NBLK, GROUP)]
    # chunk 0 as large as the int16 gather-index range allows (NC*ln<=32768):
    # the second half's AllGathers are the ones whose latency is exposed, so
    # make them as small as possible
    max_blk0 = (32768 // NC) // P  # blocks whose rows fit half 0
    cg = max(1, min(len(groups0) - 1, max_blk0 // GROUP))
    chunks = []
    chunk_last_group = []
    for g0 in range(0, len(groups0), cg):
        grs = groups0[g0 : g0 + cg]
        blocks = [b for gr in grs for b in gr]
        r0 = blocks[0] * P
        r1 = blocks[-1] * P + P
        chunks.append((r0, r1 - r0))
        chunk_last_group.append(g0 + len(grs) - 1)
    for r0k, lnk in chunks:
        assert NC * lnk <= 32768, "table half exceeds int16 gather index range"

    chunk_of_row = np.empty(NBP, dtype=np.int64)
    for k, (r0, ln) in enumerate(chunks):
        chunk_of_row[r0 : r0 + ln] = k
    r0_arr = np.array([c[0] for c in chunks])
    len_arr = np.array([c[1] for c in chunks])

    # table rows are p-major within a chunk: node (core c, block b, slot p)
    # of half k sits at row c*len_k + p*nblocks_k + (b - b0_k), so the
    # device-side shard store coalesces into one big descriptor per
    # SBUF partition instead of one 256B descriptor per row.
    k_of = chunk_of_row[pos_in_core]  # table half of each node
    core = np.arange(N) // NPC
    nblk_arr = len_arr // P
    slot_all = pos_in_core & 127
    blk_all = pos_in_core >> 7
    src_idx_all = core * len_arr[k_of] + slot_all * nblk_arr[k_of] + (
        blk_all - r0_arr[k_of] // P
    )

    blk = pos_in_core[col] >> 7
    dloc = pos_in_core[col] & 127
    seg = k_of[row]  # which table half the source row lives in
    src_idx = src_idx_all[row]

    key = (core[col] * NBLK + blk) * 2 + seg
    order = np.argsort(key, kind="stable")
    skey = key[order]
    ssrc = src_idx[order]
    sdloc = dloc[order]
    nbuck = NC * NBLK * 2
    starts = np.searchsorted(skey, np.arange(nbuck))
    ends = np.searchsorted(skey, np.arange(nbuck) + 1)
    cnt = (ends - starts).reshape(NC, NBLK, 2)

    T_LO = np.maximum(1, np.ceil(cnt[:, :, 0] / P).max(axis=0).astype(np.int64))
    T_HI = np.ceil(cnt[:, :, 1] / P).max(axis=0).astype(np.int64)

    groups = groups0
    T_consume = int((T_LO + T_HI).sum())

    in_maps = []
    w1bf = np.asarray(W1, dtype=ml_dtypes.bfloat16)
    w2bf = np.asarray(W2, dtype=ml_dtypes.bfloat16)
    b1f = np.asarray(b1, dtype=np.float32)
    b2f = np.asarray(b2, dtype=np.float32)
    b1b = np.broadcast_to(b1f[None, :], (P, Fh)).copy()
    b2c = np.ascontiguousarray(b2f[:, None])  # [Fout, 1]

    xT_cores = []
    for c in range(NC):
        # did columns: all lo tiles in block order, then all hi tiles, so a
        # whole group's selectors per phase build in ONE is_equal op
        did = np.full((P, T_consume), PAD_DEST, dtype=np.float32)
        TSEG = (T_LO, T_HI)
        ccol = 0
        for sg in (0, 1):
            for b in range(NBLK):
                bidx = (c * NBLK + b) * 2 + sg
                n = ends[bidx] - starts[bidx]
                T = int(TSEG[sg][b])
                if T == 0:
                    assert n == 0
                    continue
                tmp = np.full(T * P, PAD_DEST, dtype=np.float32)
                tmp[:n] = sdloc[starts[bidx] : ends[bidx]]
                did[:, ccol : ccol + T] = tmp.reshape(T, P).T
                ccol += T
        assert ccol == T_consume

        idx_cols = []
        for blocks in groups:
            for sg in (0, 1):
                for b in blocks:
                    T = int(TSEG[sg][b])
                    if T == 0:
                        continue
                    bidx = (c * NBLK + b) * 2 + sg
                    n = ends[bidx] - starts[bidx]
                    s = ssrc[starts[bidx] : ends[bidx]]
                    tmp = np.zeros(T * P, dtype=np.int64)
                    tmp[:n] = s
                    idx_cols.append(_pack_idx(tmp))
        idx = (
            np.concatenate(idx_cols, axis=1)
            if idx_cols
            else np.zeros((P, 8), np.int16)
        )

        # dinv and dinv^2 columns at permuted positions (pad 1.0)
        dvflat = np.ones(NBP, dtype=np.float32)
        dvflat[pos_in_core[c * NPC : (c + 1) * NPC]] = dinv[c * NPC : (c + 1) * NPC]
        dvc = np.ascontiguousarray(dvflat.reshape(NBLK, P).T)
        dv2c = np.ascontiguousarray(dvc * dvc)

        # x rows permuted, dinv-scaled, transposed: [Fin, NBP] bf16
        xsc = np.asarray(x)[node_at[c]] * dinv[node_at[c]][:, None]
        xT = np.zeros((Fin, NBP), dtype=ml_dtypes.bfloat16)
        xT[:, :NPC] = xsc.T.astype(ml_dtypes.bfloat16)
        xT_cores.append(xT)

        in_maps.append(
            {
                "xTf": None,  # filled below (shared across cores)
                "cid": np.array([[c]], dtype=np.int32),
                "w1": w1bf,
                "w2": w2bf,
                "b1b": b1b,
                "b2c": b2c,
                "dv": dvc,
                "dv2": dv2c,
                "idx": np.ascontiguousarray(idx),
                "did": did.astype(ml_dtypes.bfloat16),
            }
        )

    xTf = np.ascontiguousarray(np.concatenate(xT_cores, axis=1))
    for m in in_maps:
        m["xTf"] = xTf

    meta = dict(
        N=N,
        Fin=Fin,
        Fh=Fh,
        Fout=Fout,
        NPC=NPC,
        NBLK=NBLK,
        NBP=NBP,
        T_LO=[int(t) for t in T_LO],
        T_HI=[int(t) for t in T_HI],
        T_MAX=int(max(int(T_LO[b]) + int(T_HI[b]) for b in range(NBLK))),
        TG_MAX=int(max(
            max(sum(int(T_LO[b]) for b in g), sum(int(T_HI[b]) for b in g))
            for g in groups0
        )),
        groups=groups,
        chunks=chunks,
        chunk_last_group=chunk_last_group,
        idx_w=in_maps[0]["idx"].shape[1],
        T_consume=T_consume,
        has_b1=bool(np.any(b1f)),
    )
    return in_maps, meta, node_at


def _build_program(meta):
    import concourse.bacc as bacc
    import concourse.bass as bass
    import concourse.mybir as mybir
    import concourse.tile as tile
    from concourse.masks import make_identity

    Fin, Fh, Fout = meta["Fin"], meta["Fh"], meta["Fout"]
    NPC, NBLK, NBP = meta["NPC"], meta["NBLK"], meta["NBP"]
    T_LO, T_HI = meta["T_LO"], meta["T_HI"]
    groups = meta["groups"]
    chunks = meta["chunks"]
    TW = ROWW * Fh  # table row width in bf16 elements

    f32 = mybir.dt.float32
    bf16 = mybir.dt.bfloat16
    i16 = mybir.dt.int16
    i32 = mybir.dt.int32

    nc = bacc.Bacc(
        "TRN2", target_bir_lowering=False, debug=True, num_swdge_queues=NQUEUES
    )

    xTf = nc.dram_tensor("xTf", [Fin, NC * NBP], bf16, kind="ExternalInput")
    cidd = nc.dram_tensor("cid", [1, 1], i32, kind="ExternalInput")
    w1 = nc.dram_tensor("w1", [Fin, Fh], bf16, kind="ExternalInput")
    w2 = nc.dram_tensor("w2", [Fh, Fout], bf16, kind="ExternalInput")
    b1b = nc.dram_tensor("b1b", [P, Fh], f32, kind="ExternalInput")
    b2c = nc.dram_tensor("b2c", [Fout, 1], f32, kind="ExternalInput")
    dv = nc.dram_tensor("dv", [P, NBLK], f32, kind="ExternalInput")
    dv2 = nc.dram_tensor("dv2", [P, NBLK], f32, kind="ExternalInput")
    idxd = nc.dram_tensor("idx", [P, meta["idx_w"]], i16, kind="ExternalInput")
    didd = nc.dram_tensor("did", [P, meta["T_consume"]], bf16, kind="ExternalInput")
    out = nc.dram_tensor("out", [Fout, NPC], f32, kind="ExternalOutput")

    qctr = [0]

    with tile.TileContext(nc, num_cores=NC) as tc, ExitStack() as ctx:
        consts = ctx.enter_context(tc.tile_pool(name="consts", bufs=1))
        dram = ctx.enter_context(tc.tile_pool(name="dram", bufs=1, space="DRAM"))
        wpool = ctx.enter_context(tc.tile_pool(name="work", bufs=4))
        selp = ctx.enter_context(tc.tile_pool(name="sel", bufs=3))
        glo = ctx.enter_context(tc.tile_pool(name="glo", bufs=4))
        ghi = ctx.enter_context(tc.tile_pool(name="ghi", bufs=3))
        obat = ctx.enter_context(tc.tile_pool(name="obat", bufs=2))
        pg = ctx.enter_context(tc.tile_pool(name="pg", bufs=4, space="PSUM"))
        pt = ctx.enter_context(tc.tile_pool(name="pt", bufs=1, space="PSUM"))
        ph = ctx.enter_context(tc.tile_pool(name="ph", bufs=1, space="PSUM"))
        pq = ctx.enter_context(tc.tile_pool(name="pq", bufs=2, space="PSUM"))
        xfp = ctx.enter_context(tc.tile_pool(name="xfp", bufs=2))
        stg = ctx.enter_context(tc.tile_pool(name="stg", bufs=2))

        # ---- constants
        ident = consts.tile([P, P], f32)
        make_identity(nc, ident[:])
        ident_bf = consts.tile([P, P], bf16)
        nc.vector.tensor_copy(ident_bf[:], ident[:])
        # bf16 iota/did: values are small exact integers; 16-bit inputs get
        # 2x DVE throughput on the per-group is_equal selector build. A
        # single 0..127 column row is broadcast across the tile dim.
        iota_i = wpool.tile([P, 1, P], i32, tag="iota_i")
        nc.gpsimd.iota(
            iota_i[:], pattern=[[0, 1], [1, P]], base=0, channel_multiplier=0
        )
        iota_f = consts.tile([P, 1, P], bf16)
        nc.vector.tensor_copy(iota_f[:], iota_i[:])
        w1t = consts.tile([Fin, Fh], bf16)
        nc.sync.dma_start(w1t[:], w1[:])
        w2t = consts.tile([Fh, Fout], bf16)
        nc.sync.dma_start(w2t[:], w2[:])
        dvt = consts.tile([P, NBLK], f32)
        nc.sync.dma_start(dvt[:], dv[:])
        dv2t = consts.tile([P, NBLK], f32)
        nc.sync.dma_start(dv2t[:], dv2[:])
        didt = consts.tile([P, meta["T_consume"]], bf16)
        nc.sync.dma_start(didt[:], didd[:])
        idxt = consts.tile([P, meta["idx_w"]], i16)
        nc.sync.dma_start(idxt[:], idxd[:])
        b1t = consts.tile([P, Fh], f32)
        nc.sync.dma_start(b1t[:], b1b[:])
        b2t = consts.tile([Fout, 1], f32)
        nc.sync.dma_start(b2t[:], b2c[:])
        # runtime core id (per-core input), used to slice this core's own
        # rows (hall) out of the locally-computed full table. Loaded via
        # SBUF; runtime bounds-check asserts crash this runtime.
        cidt = consts.tile([1, 1], i32)
        nc.sync.dma_start(cidt[:], cidd[:])
        cid = nc.values_load(
            cidt[0:1, 0:1], engines=[mybir.EngineType.SP],
            min_val=0, max_val=NC - 1, skip_runtime_bounds_check=True,
        )

        # core-resident table shards: hall = h' rows, hall2 = h2pre rows
        hall = consts.tile([P, NBLK, Fh], bf16)
        hall2 = consts.tile([P, NBLK, Fh], bf16)


        # layer-1 gather tables are computed LOCALLY on every core (the
        # replicated-x full sweep below) — no AllGather for layer 1 at all.
        t1loc = [
            dram.tile([NC * ln, TW], bf16, name=f"t1loc{k}")
            for k, (_, ln) in enumerate(chunks)
        ]
        # layer 2 still needs cross-core transport: local shard staging +
        # shared gather tables written by one AllGather per half
        h2s = dram.tile([NBP, TW], bf16)
        t2h = [
            dram.tile([NC * ln, TW], bf16, addr_space="Shared", name=f"t2h{k}")
            for k, (_, ln) in enumerate(chunks)
        ]

        def store_chunk(shard, src_hall, k):
            """Store chunk k's blocks of src_hall into the local shard in
            p-major row order: one contiguous multi-row descriptor per
            partition instead of one 256B descriptor per table row."""
            r0, ln = chunks[k]
            b0, nb = r0 // P, ln // P
            nc.sync.dma_start(
                shard[r0 : r0 + ln, 0:Fh].rearrange("(p b) f -> p b f", b=nb),
                src_hall[:, b0 : b0 + nb, 0:Fh],
            )

        def ag_chunk(shard, halves, k):
            r0, ln = chunks[k]
            nc.gpsimd.collective_compute(
                "AllGather",
                mybir.AluOpType.bypass,
                replica_groups=[list(range(NC))],
                ins=[shard[r0 : r0 + ln, :]],
                outs=[halves[k].opt()],
            )

        chunk0_last_block = (chunks[0][0] + chunks[0][1]) // P - 1

        # ---- phase A: every core computes the WHOLE layer-1 table
        # (redundant compute beats AllGather latency): per (core-slice,
        # half), stream xTf columns in, matmul into quad-PSUM, cast on
        # alternating engines, store p-major. This core's own rows (hall,
        # for the self-loop matmuls) are then sliced back out of the table
        # with a DynSlice row offset.
        cast_flip = [0]
        for k, (r0, ln) in enumerate(chunks):
            b0, nb = r0 // P, ln // P
            for c2 in range(NC):
                xp = xfp.tile([P, nb * P], bf16, tag="xp")
                nc.sync.dma_start(
                    xp[:], xTf[:, c2 * NBP + r0 : c2 * NBP + r0 + ln]
                )
                stgt = stg.tile([P, nb, Fh], bf16, tag="stg")
                for q in range(0, nb, 4):
                    qn = min(4, nb - q)
                    pqt = pq.tile([P, 4 * Fh], f32, tag="pq")
                    for j in range(qn):
                        nc.tensor.matmul(
                            pqt[:, j * Fh : (j + 1) * Fh],
                            lhsT=xp[:, (q + j) * P : (q + j + 1) * P],
                            rhs=w1t[:], start=True, stop=True,
                        )
                    if cast_flip[0] % 2 == 0:
                        nc.scalar.activation(
                            stgt[:, q : q + qn, 0:Fh],
                            pqt[:, 0 : qn * Fh],
                            mybir.ActivationFunctionType.Copy,
                        )
                    else:
                        nc.vector.tensor_copy(
                            stgt[:, q : q + qn, 0:Fh], pqt[:, 0 : qn * Fh]
                        )
                    cast_flip[0] += 1
                nc.scalar.dma_start(
                    t1loc[k][c2 * ln : (c2 + 1) * ln, 0:Fh].rearrange(
                        "(p b) f -> p b f", b=nb
                    ),
                    stgt[:],
                )
            # own rows of this half -> hall
            nc.sync.dma_start(
                hall[:, b0 : b0 + nb, 0:Fh],
                t1loc[k][bass.ds(cid * ln, ln), 0:Fh].rearrange(
                    "(p b) f -> p b f", b=nb
                ),
            )

        def run_layer(layer):
            halves = t1loc if layer == 1 else t2h
            lo_ap = halves[0][:]
            hi_ap = halves[1][:] if len(halves) > 1 else None
            src_hall = hall if layer == 1 else hall2

            def chunked_gather(buf, src_ap, ntiles, idx_off):
                c0 = 0
                while c0 < ntiles:
                    cn = min(MAXCALL, ntiles - c0)
                    nc.gpsimd.dma_gather(
                        buf[:, c0 : c0 + cn, :], src_ap,
                        idxt[:, 8 * (idx_off + c0) : 8 * (idx_off + c0 + cn)],
                        P * cn, P * cn, TW,
                        queue_num=qctr[0] % NQUEUES,
                    )
                    qctr[0] += 1
                    c0 += cn

            # packed-idx tile offsets (gather order: per group, lo then hi);
            # both layers gather with the SAME indices. did columns are
            # packed [all lo tiles in block order | all hi tiles].
            offs = []
            o = 0
            for blocks in groups:
                tlo_g = sum(T_LO[b] for b in blocks)
                thi_g = sum(T_HI[b] for b in blocks)
                offs.append((o, o + tlo_g, tlo_g, thi_g))
                o += tlo_g + thi_g
            dbase_lo = []
            o = 0
            for b in range(NBLK):
                dbase_lo.append(o)
                o += T_LO[b]
            dbase_hi = []
            for b in range(NBLK):
                dbase_hi.append(o)
                o += T_HI[b]

            def build_sel(cols, ntile):
                sel = selp.tile([P, ntile, P], bf16, tag="sel")
                nc.vector.tensor_tensor(
                    sel[:],
                    didt[:, cols : cols + ntile].to_broadcast([P, ntile, P]),
                    iota_f[:].to_broadcast([P, ntile, P]),
                    mybir.AluOpType.is_equal,
                )
                return sel

            lobs = [None] * len(groups)

            def emit_lo(gk):
                lo_off, hi_off, tlo_g, thi_g = offs[gk]
                lob = glo.tile([P, tlo_g, TW], bf16, tag="glo")
                chunked_gather(lob, lo_ap, tlo_g, lo_off)
                lobs[gk] = lob

            def emit_hi_and_consume(gk, blocks):
                lo_off, hi_off, tlo_g, thi_g = offs[gk]
                hib = None
                if thi_g > 0:
                    hib = ghi.tile([P, thi_g, TW], bf16, tag="ghi")
                    chunked_gather(hib, hi_ap, thi_g, hi_off)
                consume(gk, blocks, lobs[gk], hib)

            def consume(gk, blocks, lob, hib):
                # two selector builds per group: one for its lo tiles, one
                # for its hi tiles (did is packed lo-block-major | hi)
                tlo_g = sum(T_LO[b] for b in blocks)
                thi_g = sum(T_HI[b] for b in blocks)
                sel_lo = build_sel(dbase_lo[blocks[0]], tlo_g)
                sel_hi = build_sel(dbase_hi[blocks[0]], thi_g) if thi_g else None
                lo_t = 0
                hi_t = 0
                for b in blocks:
                    nv = min(P, NPC - b * P)

                    acc = pg.tile([P, Fh], f32, tag="pg")
                    # own rows: the reference's added self-loop, via identity
                    nc.tensor.matmul(
                        acc[:], lhsT=ident_bf[:], rhs=src_hall[:, b, 0:Fh],
                        start=True, stop=False,
                    )
                    nmm = T_LO[b] + T_HI[b]
                    k = 0
                    for sel, buf, t0, tn in (
                        (sel_lo, lob, lo_t, T_LO[b]),
                        (sel_hi, hib, hi_t, T_HI[b]),
                    ):
                        for t in range(tn):
                            nc.tensor.matmul(
                                acc[:],
                                lhsT=sel[:, t0 + t, :],
                                rhs=buf[:, t0 + t, 0:Fh],
                                start=False,
                                stop=(k == nmm - 1),
                            )
                            k += 1
                    lo_t += T_LO[b]
                    hi_t += T_HI[b]

                    if layer == 1:
                        # h2pre = dinv*relu(dinv*G + b1) = relu(dinv^2*G), b1=0
                        if meta["has_b1"]:
                            tmp = wpool.tile([P, Fh], f32, tag="l1tmp")
                            nc.vector.tensor_scalar(
                                tmp[:], acc[:], dvt[:, b : b + 1], None,
                                mybir.AluOpType.mult,
                            )
                            nc.vector.tensor_tensor(
                                tmp[:], tmp[:], b1t[:], mybir.AluOpType.add
                            )
                            nc.scalar.activation(
                                hall2[:, b, 0:Fh], tmp[:],
                                mybir.ActivationFunctionType.Relu,
                                scale=dvt[:, b : b + 1],
                            )
                        else:
                            nc.scalar.activation(
                                hall2[:, b, 0:Fh], acc[:],
                                mybir.ActivationFunctionType.Relu,
                                scale=dv2t[:, b : b + 1],
                            )
                        if b == chunk0_last_block:
                            store_chunk(h2s, hall2, 0)
                            ag_chunk(h2s, t2h, 0)
                        elif b == NBLK - 1:
                            store_chunk(h2s, hall2, 1)
                            ag_chunk(h2s, t2h, 1)
                    else:
                        # out.T[:, block] = W2.T @ (dinv*G2).T + b2
                        r2 = wpool.tile([P, Fh], f32, tag="l2r")
                        nc.scalar.activation(
                            r2[:], acc[:], mybir.ActivationFunctionType.Copy,
                            scale=dvt[:, b : b + 1],
                        )
                        r2T_ps = pt.tile([P, P], f32, tag="pt")
                        nc.tensor.transpose(r2T_ps[:], r2[:], ident[:])
                        r2T = wpool.tile([P, P], bf16, tag="wbf")
                        nc.vector.tensor_copy(r2T[:], r2T_ps[:])
                        o2T_ps = ph.tile([Fout, P], f32, tag="ph")
                        nc.tensor.matmul(
                            o2T_ps[:], lhsT=w2t[:], rhs=r2T[:],
                            start=True, stop=True,
                        )
                        bi = b % STORE_BATCH
                        if bi == 0:
                            obt = obat.tile([Fout, STORE_BATCH, P], f32,
                                            tag="obt")
                            obts[0] = obt
                        nc.scalar.activation(
                            obts[0][:, bi, :], o2T_ps[:],
                            mybir.ActivationFunctionType.Identity,
                            bias=b2t[:, 0:1],
                        )
                        if bi == STORE_BATCH - 1 or b == NBLK - 1:
                            b0 = b - bi
                            ncols = min(NPC, (b + 1) * P) - b0 * P
                            nc.sync.dma_start(
                                out[:, b0 * P : b0 * P + ncols],
                                obts[0][:, 0 : bi + 1, :].rearrange(
                                    "f b p -> f (b p)"
                                )[:, 0:ncols],
                            )

            obts = [None]

            # software-pipelined emission: lo gathers run LA groups ahead so
            # a hi call's wait on the second table half's AllGather never
            # starves the gpsimd queue of runnable lo gathers.
            LA = 2 if layer == 1 else 3
            for i in range(min(LA, len(groups))):
                emit_lo(i)
            for gk, blocks in enumerate(groups):
                emit_hi_and_consume(gk, blocks)
                if gk + LA < len(groups):
                    emit_lo(gk + LA)

        run_layer(1)
        run_layer(2)

    nc.compile()
    return nc


def _assemble(results, meta, node_at):
    N, Fout = meta["N"], meta["Fout"]
    out = np.empty((N, Fout), dtype=np.float32)
    for c in range(NC):
        out[node_at[c]] = np.asarray(results[c]["out"]).T
    return out


def kernel(**inputs) -> np.ndarray:
    x = np.asarray(inputs["x"])
    edge_index = np.asarray(inputs["edge_index"])
    W1 = np.asarray(inputs["W1"])
    b1 = np.asarray(inputs["b1"])
    W2 = np.asarray(inputs["W2"])
    b2 = np.asarray(inputs["b2"])

    in_maps, meta, node_at = _preprocess(x, edge_index, W1, b1, W2, b2)
    nc = _build_program(meta)

    from concourse.bass_utils import run_bass_kernel_spmd

    res = run_bass_kernel_spmd(nc, in_maps, list(range(NC)))
    return _assemble(res.results, meta, node_at)


# revision 32
# speedup vs baseline: 1.0943x; 1.0943x over previous
"""Two-layer GCN (PyG GCNConv x2, eval mode) on 8 Trainium2 NeuronCores.

out = S @ (relu(S @ (x@W1) + b1) @ W2) + b2,  S = D^-1/2 (A+I) D^-1/2

Design (v14, destination-sharded, local layer-1 table):
  - 50000 nodes sharded 6250/core (padded to 6272 = 49 blocks of 128);
    per core, destinations are permuted into blocks balanced by in-degree
    (host permutation, undone on the way out).
  - The 8 "cores" are 8 separate devices with NO shared HBM: all
    cross-core data must move through AllGather collectives.
  - Layer-1 table: every core computes the WHOLE h' = dinv*(x@W1) table
    from a replicated, host-pre-scaled-and-transposed x (redundant
    compute beats AllGather latency): streamed column pieces, matmuls
    into quad-PSUM banks, f32->bf16 casts alternating between ScalarE
    and VectorE, stored p-major so each piece is one descriptor per
    partition. This core's own rows (self-loop operand `hall`) are
    sliced back out with a runtime-core-id DynSlice.
  - Layer-2 table: h2pre shards are AllGathered into Shared-DRAM tables
    (one collective per table half; the sim enforces a single writer
    inst per Shared tensor).
  - Table rows are 256B (128 bf16 cols, no pad). int16 gather indices
    force a lo/hi table-half split; half 0 is sized to the int16 max
    (32 blocks/core) so the latency-exposed second-half AllGathers are
    as small as possible. Rows are p-major within a (core, half) slice.
  - Aggregation per 128-dest block: dma_gather source rows (<=1024 idx
    per call - the Q7 ucode dies above that - round-robined over 4 SWDGE
    queues), one-hot selectors built per GROUP of 4 blocks in one
    broadcast-AP is_equal (bf16 in/out for 2x DVE rate; did is packed
    [all lo tiles | all hi tiles] to keep group columns contiguous),
    then matmul(lhsT=onehot[e,d], rhs=msgs[e,f]) accumulating in PSUM;
    an identity-selector matmul adds the block's own rows, so the
    reference's self-loops never touch the gather path.
  - Gather emission is software-pipelined: 2 lo-groups run ahead so a hi
    call's wait on the second table half never starves the gpsimd queue.
  - Layer 1 tail: h2pre = dinv*relu(dinv*G) = relu(dinv^2*G) in one
    ScalarE op (b1==0) into hall2; layer 2 applies W2 after aggregation:
    out = dinv*(G2)@W2 + b2, emitted feature-major, transposed on host.
"""

import math
from contextlib import ExitStack

import numpy as np

NC = 8
P = 128
GROUP = 4  # dest blocks per gather buffer group
MAXCALL = 8  # tiles per dma_gather call (1024 idx ucode limit)
NQUEUES = 4
PAD_DEST = 200  # destid for padding edges; never matches iota 0..127
ROWW = 1  # table row width multiplier: 1 => 256B rows, 2 => 512B rows
STORE_BATCH = 7  # dest blocks per table-store DMA (49 = 7*7)


def _pack_idx(v: np.ndarray) -> np.ndarray:
    """[T*128] int -> [128, 8T] int16 in dma_gather's wrap-16 layout,
    replicated over the 8 gpsimd cores (element i lives at [i%16, i//16])."""
    assert v.size % P == 0
    a = v.reshape(-1, 16).T.astype(np.int16)  # [16, 8T]
    return np.tile(a, (8, 1))  # [128, 8T]


def _balance_blocks(weights: np.ndarray, nblk: int) -> np.ndarray:
    """Assign len(weights) items into nblk blocks of <=128, balancing block
    weight sums. Returns pos[i] = block*128 + slot."""
    import heapq

    n = weights.size
    order = np.argsort(-weights, kind="stable")
    loads = np.zeros(nblk, dtype=np.int64)
    fill = np.zeros(nblk, dtype=np.int64)
    cap = np.full(nblk, P, dtype=np.int64)
    cap[nblk - 1] = n - (nblk - 1) * P  # last block holds the remainder
    pos = np.empty(n, dtype=np.int64)
    heap = [(0, b) for b in range(nblk)]
    heapq.heapify(heap)
    for i in order:
        while True:
            load, b = heapq.heappop(heap)
            if fill[b] < cap[b]:
                break
        pos[i] = b * P + fill[b]
        fill[b] += 1
        loads[b] = load + weights[i]
        if fill[b] < cap[b]:
            heapq.heappush(heap, (int(loads[b]), b))
    return pos


def _preprocess(x, edge_index, W1, b1, W2, b2):
    import ml_dtypes

    N, Fin = x.shape
    Fh = W1.shape[1]
    Fout = W2.shape[1]
    assert N % NC == 0
    NPC = N // NC
    NBLK = math.ceil(NPC / P)
    NBP = NBLK * P  # padded rows per core in the gather tables

    row = np.asarray(edge_index[0], dtype=np.int64)
    col = np.asarray(edge_index[1], dtype=np.int64)

    # degrees include the self-loop the reference appends to every node
    deg = (np.bincount(col, minlength=N) + 1).astype(np.float64)
    dinv = (1.0 / np.sqrt(deg)).astype(np.float32)

    # per-core balanced permutation of destination slots by in-edge count
    cnt_in = np.bincount(col, minlength=N)
    pos_in_core = np.empty(N, dtype=np.int64)
    node_at = np.empty((NC, NPC), dtype=np.int64)
    for c in range(NC):
        w = cnt_in[c * NPC : (c + 1) * NPC]
        p = _balance_blocks(w, NBLK)
        pos_in_core[c * NPC : (c + 1) * NPC] = p
        node_at[c][p] = np.arange(NPC) + c * NPC

    # two table halves (int16 gather index limit: NC*ln <= 32768 rows);
    # half k covers core-local padded rows [r0,r1) (block-aligned); the
    # table holds the 8 cores' slices consecutively: node (c, p, half k)
    # lives at table-k row c*len_k + (p - r0_k).
    groups0 = [list(range(g, min(g + GROUP, NBLK))) for g in range(0, NBLK, GROUP)]
    # chunk 0 as large as the int16 gather-index range allows (NC*ln<=32768):
    # the second half's AllGathers are the ones whose latency is exposed, so
    # make them as small as possible
    max_blk0 = (32768 // NC) // P  # blocks whose rows fit half 0
    cg = max(1, min(len(groups0) - 1, max_blk0 // GROUP))
    chunks = []
    chunk_last_group = []
    for g0 in range(0, len(groups0), cg):
        grs = groups0[g0 : g0 + cg]
        blocks = [b for gr in grs for b in gr]
        r0 = blocks[0] * P
        r1 = blocks[-1] * P + P
        chunks.append((r0, r1 - r0))
        chunk_last_group.append(g0 + len(grs) - 1)
    for r0k, lnk in chunks:
        assert NC * lnk <= 32768, "table half exceeds int16 gather index range"

    chunk_of_row = np.empty(NBP, dtype=np.int64)
    for k, (r0, ln) in enumerate(chunks):
        chunk_of_row[r0 : r0 + ln] = k
    r0_arr = np.array([c[0] for c in chunks])
    len_arr = np.array([c[1] for c in chunks])

    # table rows are p-major within a chunk: node (core c, block b, slot p)
    # of half k sits at row c*len_k + p*nblocks_k + (b - b0_k), so the
    # device-side shard store coalesces into one big descriptor per
    # SBUF partition instead of one 256B descriptor per row.
    k_of = chunk_of_row[pos_in_core]  # table half of each node
    core = np.arange(N) // NPC
    nblk_arr = len_arr // P
    slot_all = pos_in_core & 127
    blk_all = pos_in_core >> 7
    src_idx_all = core * len_arr[k_of] + slot_all * nblk_arr[k_of] + (
        blk_all - r0_arr[k_of] // P
    )

    blk = pos_in_core[col] >> 7
    dloc = pos_in_core[col] & 127
    seg = k_of[row]  # which table half the source row lives in
    src_idx = src_idx_all[row]

    key = (core[col] * NBLK + blk) * 2 + seg
    order = np.argsort(key, kind="stable")
    skey = key[order]
    ssrc = src_idx[order]
    sdloc = dloc[order]
    nbuck = NC * NBLK * 2
    starts = np.searchsorted(skey, np.arange(nbuck))
    ends = np.searchsorted(skey, np.arange(nbuck) + 1)
    cnt = (ends - starts).reshape(NC, NBLK, 2)

    T_LO = np.maximum(1, np.ceil(cnt[:, :, 0] / P).max(axis=0).astype(np.int64))
    T_HI = np.ceil(cnt[:, :, 1] / P).max(axis=0).astype(np.int64)

    groups = groups0
    T_consume = int((T_LO + T_HI).sum())

    in_maps = []
    w1bf = np.asarray(W1, dtype=ml_dtypes.bfloat16)
    w2bf = np.asarray(W2, dtype=ml_dtypes.bfloat16)
    b1f = np.asarray(b1, dtype=np.float32)
    b2f = np.asarray(b2, dtype=np.float32)
    b1b = np.broadcast_to(b1f[None, :], (P, Fh)).copy()
    b2c = np.ascontiguousarray(b2f[:, None])  # [Fout, 1]

    xT_cores = []
    for c in range(NC):
        # did columns: all lo tiles in block order, then all hi tiles, so a
        # whole group's selectors per phase build in ONE is_equal op
        did = np.full((P, T_consume), PAD_DEST, dtype=np.float32)
        TSEG = (T_LO, T_HI)
        ccol = 0
        for sg in (0, 1):
            for b in range(NBLK):
                bidx = (c * NBLK + b) * 2 + sg
                n = ends[bidx] - starts[bidx]
                T = int(TSEG[sg][b])
                if T == 0:
                    assert n == 0
                    continue
                tmp = np.full(T * P, PAD_DEST, dtype=np.float32)
                tmp[:n] = sdloc[starts[bidx] : ends[bidx]]
                did[:, ccol : ccol + T] = tmp.reshape(T, P).T
                ccol += T
        assert ccol == T_consume

        idx_cols = []
        for blocks in groups:
            for sg in (0, 1):
                for b in blocks:
                    T = int(TSEG[sg][b])
                    if T == 0:
                        continue
                    bidx = (c * NBLK + b) * 2 + sg
                    n = ends[bidx] - starts[bidx]
                    s = ssrc[starts[bidx] : ends[bidx]]
                    tmp = np.zeros(T * P, dtype=np.int64)
                    tmp[:n] = s
                    idx_cols.append(_pack_idx(tmp))
        idx = (
            np.concatenate(idx_cols, axis=1)
            if idx_cols
            else np.zeros((P, 8), np.int16)
        )

        # dinv and dinv^2 columns at permuted positions (pad 1.0)
        dvflat = np.ones(NBP, dtype=np.float32)
        dvflat[pos_in_core[c * NPC : (c + 1) * NPC]] = dinv[c * NPC : (c + 1) * NPC]
        dvc = np.ascontiguousarray(dvflat.reshape(NBLK, P).T)
        dv2c = np.ascontiguousarray(dvc * dvc)

        # x rows permuted, dinv-scaled, transposed: [Fin, NBP] bf16
        xsc = np.asarray(x)[node_at[c]] * dinv[node_at[c]][:, None]
        xT = np.zeros((Fin, NBP), dtype=ml_dtypes.bfloat16)
        xT[:, :NPC] = xsc.T.astype(ml_dtypes.bfloat16)
        xT_cores.append(xT)

        in_maps.append(
            {
                "xTf": None,  # filled below (shared across cores)
                "cid": np.array([[c]], dtype=np.int32),
                "w1": w1bf,
                "w2": w2bf,
                "b1b": b1b,
                "b2c": b2c,
                "dv": dvc,
                "dv2": dv2c,
                "idx": np.ascontiguousarray(idx),
                "did": did.astype(ml_dtypes.bfloat16),
            }
        )

    xTf = np.ascontiguousarray(np.concatenate(xT_cores, axis=1))
    for m in in_maps:
        m["xTf"] = xTf

    meta = dict(
        N=N,
        Fin=Fin,
        Fh=Fh,
        Fout=Fout,
        NPC=NPC,
        NBLK=NBLK,
        NBP=NBP,
        T_LO=[int(t) for t in T_LO],
        T_HI=[int(t) for t in T_HI],
        T_MAX=int(max(int(T_LO[b]) + int(T_HI[b]) for b in range(NBLK))),
        TG_MAX=int(max(
            max(sum(int(T_LO[b]) for b in g), sum(int(T_HI[b]) for b in g))
            for g in groups0
        )),
        groups=groups,
        chunks=chunks,
        chunk_last_group=chunk_last_group,
        idx_w=in_maps[0]["idx"].shape[1],
        T_consume=T_consume,
        has_b1=bool(np.any(b1f)),
    )
    return in_maps, meta, node_at


def _build_program(meta):
    import concourse.bacc as bacc
    import concourse.bass as bass
    import concourse.mybir as mybir
    import concourse.tile as tile
    from concourse.masks import make_identity

    Fin, Fh, Fout = meta["Fin"], meta["Fh"], meta["Fout"]
    NPC, NBLK, NBP = meta["NPC"], meta["NBLK"], meta["NBP"]
    T_LO, T_HI = meta["T_LO"], meta["T_HI"]
    groups = meta["groups"]
    chunks = meta["chunks"]
    TW = ROWW * Fh  # table row width in bf16 elements

    f32 = mybir.dt.float32
    bf16 = mybir.dt.bfloat16
    i16 = mybir.dt.int16
    i32 = mybir.dt.int32

    nc = bacc.Bacc(
        "TRN2", target_bir_lowering=False, debug=True, num_swdge_queues=NQUEUES
    )

    xTf = nc.dram_tensor("xTf", [Fin, NC * NBP], bf16, kind="ExternalInput")
    cidd = nc.dram_tensor("cid", [1, 1], i32, kind="ExternalInput")
    w1 = nc.dram_tensor("w1", [Fin, Fh], bf16, kind="ExternalInput")
    w2 = nc.dram_tensor("w2", [Fh, Fout], bf16, kind="ExternalInput")
    b1b = nc.dram_tensor("b1b", [P, Fh], f32, kind="ExternalInput")
    b2c = nc.dram_tensor("b2c", [Fout, 1], f32, kind="ExternalInput")
    dv = nc.dram_tensor("dv", [P, NBLK], f32, kind="ExternalInput")
    dv2 = nc.dram_tensor("dv2", [P, NBLK], f32, kind="ExternalInput")
    idxd = nc.dram_tensor("idx", [P, meta["idx_w"]], i16, kind="ExternalInput")
    didd = nc.dram_tensor("did", [P, meta["T_consume"]], bf16, kind="ExternalInput")
    out = nc.dram_tensor("out", [Fout, NPC], f32, kind="ExternalOutput")

    qctr = [0]

    with tile.TileContext(nc, num_cores=NC) as tc, ExitStack() as ctx:
        consts = ctx.enter_context(tc.tile_pool(name="consts", bufs=1))
        dram = ctx.enter_context(tc.tile_pool(name="dram", bufs=1, space="DRAM"))
        wpool = ctx.enter_context(tc.tile_pool(name="work", bufs=4))
        selp = ctx.enter_context(tc.tile_pool(name="sel", bufs=3))
        glo = ctx.enter_context(tc.tile_pool(name="glo", bufs=4))
        ghi = ctx.enter_context(tc.tile_pool(name="ghi", bufs=3))
        obat = ctx.enter_context(tc.tile_pool(name="obat", bufs=2))
        pg = ctx.enter_context(tc.tile_pool(name="pg", bufs=4, space="PSUM"))
        pt = ctx.enter_context(tc.tile_pool(name="pt", bufs=1, space="PSUM"))
        ph = ctx.enter_context(tc.tile_pool(name="ph", bufs=1, space="PSUM"))
        pq = ctx.enter_context(tc.tile_pool(name="pq", bufs=2, space="PSUM"))
        xfp = ctx.enter_context(tc.tile_pool(name="xfp", bufs=2))
        stg = ctx.enter_context(tc.tile_pool(name="stg", bufs=2))

        # ---- constants
        ident = consts.tile([P, P], f32)
        make_identity(nc, ident[:])
        ident_bf = consts.tile([P, P], bf16)
        nc.vector.tensor_copy(ident_bf[:], ident[:])
        # bf16 iota/did: values are small exact integers; 16-bit inputs get
        # 2x DVE throughput on the per-group is_equal selector build. A
        # single 0..127 column row is broadcast across the tile dim.
        iota_i = wpool.tile([P, 1, P], i32, tag="iota_i")
        nc.gpsimd.iota(
            iota_i[:], pattern=[[0, 1], [1, P]], base=0, channel_multiplier=0
        )
        iota_f = consts.tile([P, 1, P], bf16)
        nc.vector.tensor_copy(iota_f[:], iota_i[:])
        w1t = consts.tile([Fin, Fh], bf16)
        nc.sync.dma_start(w1t[:], w1[:])
        w2t = consts.tile([Fh, Fout], bf16)
        nc.sync.dma_start(w2t[:], w2[:])
        dvt = consts.tile([P, NBLK], f32)
        nc.sync.dma_start(dvt[:], dv[:])
        dv2t = consts.tile([P, NBLK], f32)
        nc.sync.dma_start(dv2t[:], dv2[:])
        didt = consts.tile([P, meta["T_consume"]], bf16)
        nc.sync.dma_start(didt[:], didd[:])
        idxt = consts.tile([P, meta["idx_w"]], i16)
        nc.sync.dma_start(idxt[:], idxd[:])
        b1t = consts.tile([P, Fh], f32)
        nc.sync.dma_start(b1t[:], b1b[:])
        b2t = consts.tile([Fout, 1], f32)
        nc.sync.dma_start(b2t[:], b2c[:])
        # runtime core id (per-core input), used to slice this core's own
        # rows (hall) out of the locally-computed full table. Loaded via
        # SBUF; runtime bounds-check asserts crash this runtime.
        cidt = consts.tile([1, 1], i32)
        nc.sync.dma_start(cidt[:], cidd[:])
        cid = nc.values_load(
            cidt[0:1, 0:1], engines=[mybir.EngineType.SP],
            min_val=0, max_val=NC - 1, skip_runtime_bounds_check=True,
        )

        # core-resident table shards: hall = h' rows, hall2 = h2pre rows
        hall = consts.tile([P, NBLK, Fh], bf16)
        hall2 = consts.tile([P, NBLK, Fh], bf16)


        # layer-1 gather tables are computed LOCALLY on every core (the
        # replicated-x full sweep below) — no AllGather for layer 1 at all.
        t1loc = [
            dram.tile([NC * ln, TW], bf16, name=f"t1loc{k}")
            for k, (_, ln) in enumerate(chunks)
        ]
        # layer 2 still needs cross-core transport: local shard staging +
        # shared gather tables written by one AllGather per half
        h2s = dram.tile([NBP, TW], bf16)
        t2h = [
            dram.tile([NC * ln, TW], bf16, addr_space="Shared", name=f"t2h{k}")
            for k, (_, ln) in enumerate(chunks)
        ]

        def store_chunk(shard, src_hall, k):
            """Store chunk k's blocks of src_hall into the local shard in
            p-major row order: one contiguous multi-row descriptor per
            partition instead of one 256B descriptor per table row."""
            r0, ln = chunks[k]
            b0, nb = r0 // P, ln // P
            nc.sync.dma_start(
                shard[r0 : r0 + ln, 0:Fh].rearrange("(p b) f -> p b f", b=nb),
                src_hall[:, b0 : b0 + nb, 0:Fh],
            )

        def ag_chunk(shard, halves, k):
            r0, ln = chunks[k]
            nc.gpsimd.collective_compute(
                "AllGather",
                mybir.AluOpType.bypass,
                replica_groups=[list(range(NC))],
                ins=[shard[r0 : r0 + ln, :]],
                outs=[halves[k].opt()],
            )

        chunk0_last_block = (chunks[0][0] + chunks[0][1]) // P - 1

        # ---- phase A: every core computes the WHOLE layer-1 table
        # (redundant compute beats AllGather latency): per (core-slice,
        # half), stream xTf columns in, matmul into quad-PSUM, cast on
        # alternating engines, store p-major. This core's own rows (hall,
        # for the self-loop matmuls) are then sliced back out of the table
        # with a DynSlice row offset.
        cast_flip = [0]
        for k, (r0, ln) in enumerate(chunks):
            b0, nb = r0 // P, ln // P
            for c2 in range(NC):
                xp = xfp.tile([P, nb * P], bf16, tag="xp")
                nc.sync.dma_start(
                    xp[:], xTf[:, c2 * NBP + r0 : c2 * NBP + r0 + ln]
                )
                stgt = stg.tile([P, nb, Fh], bf16, tag="stg")
                for q in range(0, nb, 4):
                    qn = min(4, nb - q)
                    pqt = pq.tile([P, 4 * Fh], f32, tag="pq")
                    for j in range(qn):
                        nc.tensor.matmul(
                            pqt[:, j * Fh : (j + 1) * Fh],
                            lhsT=xp[:, (q + j) * P : (q + j + 1) * P],
                            rhs=w1t[:], start=True, stop=True,
                        )
                    if cast_flip[0] % 2 == 0:
                        nc.scalar.activation(
                            stgt[:, q : q + qn, 0:Fh],
                            pqt[:, 0 : qn * Fh],
                            mybir.ActivationFunctionType.Copy,
                        )
                    else:
                        nc.vector.tensor_copy(
                            stgt[:, q : q + qn, 0:Fh], pqt[:, 0 : qn * Fh]
                        )
                    cast_flip[0] += 1
                nc.scalar.dma_start(
                    t1loc[k][c2 * ln : (c2 + 1) * ln, 0:Fh].rearrange(
                        "(p b) f -> p b f", b=nb
                    ),
                    stgt[:],
                )
            # own rows of this half -> hall
            nc.sync.dma_start(
                hall[:, b0 : b0 + nb, 0:Fh],
                t1loc[k][bass.ds(cid * ln, ln), 0:Fh].rearrange(
                    "(p b) f -> p b f", b=nb
                ),
            )

        def run_layer(layer):
            halves = t1loc if layer == 1 else t2h
            lo_ap = halves[0][:]
            hi_ap = halves[1][:] if len(halves) > 1 else None
            src_hall = hall if layer == 1 else hall2

            def chunked_gather(buf, src_ap, ntiles, idx_off):
                c0 = 0
                while c0 < ntiles:
                    cn = min(MAXCALL, ntiles - c0)
                    nc.gpsimd.dma_gather(
                        buf[:, c0 : c0 + cn, :], src_ap,
                        idxt[:, 8 * (idx_off + c0) : 8 * (idx_off + c0 + cn)],
                        P * cn, P * cn, TW,
                        queue_num=qctr[0] % NQUEUES,
                    )
                    qctr[0] += 1
                    c0 += cn

            # packed-idx tile offsets (gather order: per group, lo then hi);
            # both layers gather with the SAME indices. did columns are
            # packed [all lo tiles in block order | all hi tiles].
            offs = []
            o = 0
            for blocks in groups:
                tlo_g = sum(T_LO[b] for b in blocks)
                thi_g = sum(T_HI[b] for b in blocks)
                offs.append((o, o + tlo_g, tlo_g, thi_g))
                o += tlo_g + thi_g
            dbase_lo = []
            o = 0
            for b in range(NBLK):
                dbase_lo.append(o)
                o += T_LO[b]
            dbase_hi = []
            for b in range(NBLK):
                dbase_hi.append(o)
                o += T_HI[b]

            def build_sel(cols, ntile):
                sel = selp.tile([P, ntile, P], bf16, tag="sel")
                nc.vector.tensor_tensor(
                    sel[:],
                    didt[:, cols : cols + ntile].to_broadcast([P, ntile, P]),
                    iota_f[:].to_broadcast([P, ntile, P]),
                    mybir.AluOpType.is_equal,
                )
                return sel

            lobs = [None] * len(groups)

            def emit_lo(gk):
                lo_off, hi_off, tlo_g, thi_g = offs[gk]
                lob = glo.tile([P, tlo_g, TW], bf16, tag="glo")
                chunked_gather(lob, lo_ap, tlo_g, lo_off)
                lobs[gk] = lob

            def emit_hi_and_consume(gk, blocks):
                lo_off, hi_off, tlo_g, thi_g = offs[gk]
                hib = None
                if thi_g > 0:
                    hib = ghi.tile([P, thi_g, TW], bf16, tag="ghi")
                    chunked_gather(hib, hi_ap, thi_g, hi_off)
                consume(gk, blocks, lobs[gk], hib)

            def consume(gk, blocks, lob, hib):
                # two selector builds per group: one for its lo tiles, one
                # for its hi tiles (did is packed lo-block-major | hi)
                tlo_g = sum(T_LO[b] for b in blocks)
                thi_g = sum(T_HI[b] for b in blocks)
                sel_lo = build_sel(dbase_lo[blocks[0]], tlo_g)
                sel_hi = build_sel(dbase_hi[blocks[0]], thi_g) if thi_g else None
                lo_t = 0
                hi_t = 0
                for b in blocks:
                    nv = min(P, NPC - b * P)

                    acc = pg.tile([P, Fh], f32, tag="pg")
                    # own rows: the reference's added self-loop, via identity
                    nc.tensor.matmul(
                        acc[:], lhsT=ident_bf[:], rhs=src_hall[:, b, 0:Fh],
                        start=True, stop=False,
                    )
                    nmm = T_LO[b] + T_HI[b]
                    k = 0
                    for sel, buf, t0, tn in (
                        (sel_lo, lob, lo_t, T_LO[b]),
                        (sel_hi, hib, hi_t, T_HI[b]),
                    ):
                        for t in range(tn):
                            nc.tensor.matmul(
                                acc[:],
                                lhsT=sel[:, t0 + t, :],
                                rhs=buf[:, t0 + t, 0:Fh],
                                start=False,
                                stop=(k == nmm - 1),
                            )
                            k += 1
                    lo_t += T_LO[b]
                    hi_t += T_HI[b]

                    if layer == 1:
                        # h2pre = dinv*relu(dinv*G + b1) = relu(dinv^2*G), b1=0
                        if meta["has_b1"]:
                            tmp = wpool.tile([P, Fh], f32, tag="l1tmp")
                            nc.vector.tensor_scalar(
                                tmp[:], acc[:], dvt[:, b : b + 1], None,
                                mybir.AluOpType.mult,
                            )
                            nc.vector.tensor_tensor(
                                tmp[:], tmp[:], b1t[:], mybir.AluOpType.add
                            )
                            nc.scalar.activation(
                                hall2[:, b, 0:Fh], tmp[:],
                                mybir.ActivationFunctionType.Relu,
                                scale=dvt[:, b : b + 1],
                            )
                        else:
                            nc.scalar.activation(
                                hall2[:, b, 0:Fh], acc[:],
                                mybir.ActivationFunctionType.Relu,
                                scale=dv2t[:, b : b + 1],
                            )
                        if b == chunk0_last_block:
                            store_chunk(h2s, hall2, 0)
                            ag_chunk(h2s, t2h, 0)
                        elif b == NBLK - 1:
                            store_chunk(h2s, hall2, 1)
                            ag_chunk(h2s, t2h, 1)
                    else:
                        # out.T[:, block] = W2.T @ (dinv*G2).T + b2
                        r2 = wpool.tile([P, Fh], f32, tag="l2r")
                        nc.scalar.activation(
                            r2[:], acc[:], mybir.ActivationFunctionType.Copy,
                            scale=dvt[:, b : b + 1],
                        )
                        r2T_ps = pt.tile([P, P], f32, tag="pt")
                        nc.tensor.transpose(r2T_ps[:], r2[:], ident[:])
                        r2T = wpool.tile([P, P], bf16, tag="wbf")
                        nc.vector.tensor_copy(r2T[:], r2T_ps[:])
                        o2T_ps = ph.tile([Fout, P], f32, tag="ph")
                        nc.tensor.matmul(
                            o2T_ps[:], lhsT=w2t[:], rhs=r2T[:],
                            start=True, stop=True,
                        )
                        bi = b % STORE_BATCH
                        if bi == 0:
                            obt = obat.tile([Fout, STORE_BATCH, P], f32,
                                            tag="obt")
                            obts[0] = obt
                        nc.scalar.activation(
                            obts[0][:, bi, :], o2T_ps[:],
                            mybir.ActivationFunctionType.Identity,
                            bias=b2t[:, 0:1],
                        )
                        if bi == STORE_BATCH - 1 or b == NBLK - 1:
                            b0 = b - bi
                            ncols = min(NPC, (b + 1) * P) - b0 * P
                            nc.sync.dma_start(
                                out[:, b0 * P : b0 * P + ncols],
                                obts[0][:, 0 : bi + 1, :].rearrange(
                                    "f b p -> f (b p)"
                                )[:, 0:ncols],
                            )

            obts = [None]

            # software-pipelined emission: lo gathers run LA groups ahead so
            # a hi call's wait on the second table half's AllGather never
            # starves the gpsimd queue of runnable lo gathers.
            LA = 2
            for i in range(min(LA, len(groups))):
                emit_lo(i)
            for gk, blocks in enumerate(groups):
                emit_hi_and_consume(gk, blocks)
                if gk + LA < len(groups):
                    emit_lo(gk + LA)

        run_layer(1)
        run_layer(2)

    nc.compile()
    return nc


def _assemble(results, meta, node_at):
    N, Fout = meta["N"], meta["Fout"]
    out = np.empty((N, Fout), dtype=np.float32)
    for c in range(NC):
        out[node_at[c]] = np.asarray(results[c]["out"]).T
    return out


def kernel(**inputs) -> np.ndarray:
    x = np.asarray(inputs["x"])
    edge_index = np.asarray(inputs["edge_index"])
    W1 = np.asarray(inputs["W1"])
    b1 = np.asarray(inputs["b1"])
    W2 = np.asarray(inputs["W2"])
    b2 = np.asarray(inputs["b2"])

    in_maps, meta, node_at = _preprocess(x, edge_index, W1, b1, W2, b2)
    nc = _build_program(meta)

    from concourse.bass_utils import run_bass_kernel_spmd

    res = run_bass_kernel_spmd(nc, in_maps, list(range(NC)))
    return _assemble(res.results, meta, node_at)
